# revision 13
# baseline (speedup 1.0000x reference)
"""MoE layer (8 routed experts, top-2, shared experts) on 8 Trainium2 cores,
with selective fp8 (e4m3 DoubleRow) for low-combine-weight routed pairs.

Work layout: the shared expert is split into two virtual experts A/B of
intermediate 512 (combine weight 1), giving 8192 equal-cost token-expert
pairs. Each routed pair's gate/up can run in fp8 at 2x PE throughput; the
quantization error budget allows this only for pairs with small combine
weight, so each expert's pairs are split into its top-X by weight ("hi",
bf16) and the rest ("lo", fp8 gate/up, bf16 down).

Cores 0-3 ("R"): two expert-hi groups of exactly X tokens + a 512-token
sharedB chunk — all bf16. Cores 4-7 ("S"): a 512-token sharedA chunk (bf16)
+ two expert-lo fp8 groups. X is chosen so both core types have equal cycle
counts; shared chunks tile A/B exactly, expert-hi groups are exact by
construction, so padding is limited to the fp8 group capacities.

Matmuls are weights-stationary: gate/up produce [m=128, tokens] in PSUM
(tokens on the free dim — group sizes need no 128-padding), SwiGLU applies
in that layout, and down consumes it directly as stationary — no PE
transposes. fp8 scaling: x8 = e4m3(8x), w8 = e4m3(64W), so PSUM holds 512*g;
Silu rescales by 1/512 and the remaining 512 on the up path is folded into
the per-token combine weight (w/512) applied at the PSUM drain. The Act
engine only runs Silu (no activation-table swaps).
"""

import sys

if "/opt/trn_rl_repo" not in sys.path:
    sys.path.insert(0, "/opt/trn_rl_repo")

import ml_dtypes
import numpy as np

B, S, H = 2, 1024, 2048
N = B * S
E = 8
M = 512
P = 128
KT = H // P
MT = M // P
HT = H // P
NCORES = 8
CHUNK = 512
XSCALE = 8.0      # fp8 activation pre-scale
WSCALE = 64.0     # fp8 weight pre-scale
PSCALE = XSCALE * WSCALE

_CACHE = {}


def _chunks(n):
    if n == 0:
        return []
    k = -(-n // CHUNK)
    base, rem = divmod(n, k)
    out, c0 = [], 0
    for i in range(k):
        w = base + (1 if i < rem else 0)
        out.append((c0, w))
        c0 += w
    return out


def _build_program(collectives=True, loop_n=None, groups=((205, 0), (205, 0), (512, 0))):
    """groups: tuple of (cap_hi, cap_lo) per group."""
    import concourse.mybir as mybir
    import concourse.tile as tile
    from concourse import bacc
    from contextlib import ExitStack

    f32 = mybir.dt.float32
    bf16 = mybir.dt.bfloat16
    f8 = mybir.dt.float8e4
    AF = mybir.ActivationFunctionType
    DR = mybir.MatmulPerfMode.DoubleRow

    nc = bacc.Bacc(None)
    G = len(groups)

    prm = {}
    for g, (ch, cl) in enumerate(groups):
        cap = ch + cl
        if ch:
            prm[f"x{g}"] = nc.declare_dram_parameter(f"x{g}", [P, KT * ch], bf16, isOutput=False)
            prm[f"wg{g}"] = nc.declare_dram_parameter(f"wg{g}", [P, KT * M], bf16, isOutput=False)
            prm[f"wu{g}"] = nc.declare_dram_parameter(f"wu{g}", [P, KT * M], bf16, isOutput=False)
        if cl:
            prm[f"x8_{g}"] = nc.declare_dram_parameter(f"x8_{g}", [P, KT * cl], f8, isOutput=False)
            prm[f"wg8_{g}"] = nc.declare_dram_parameter(f"wg8_{g}", [P, KT * M], f8, isOutput=False)
            prm[f"wu8_{g}"] = nc.declare_dram_parameter(f"wu8_{g}", [P, KT * M], f8, isOutput=False)
        prm[f"wd{g}"] = nc.declare_dram_parameter(f"wd{g}", [P, MT * H], bf16, isOutput=False)
        prm[f"wc{g}"] = nc.declare_dram_parameter(f"wc{g}", [P, cap], f32, isOutput=False)
        prm[f"y{g}"] = nc.declare_dram_parameter(f"y{g}", [P, HT * cap], bf16, isOutput=True)

    with tile.TileContext(nc) as tc:
        with (
            tc.tile_pool(name="sb", bufs=1) as sb,
            tc.tile_pool(name="io", bufs=2) as io,
            tc.tile_pool(name="wk", bufs=2) as wk,
            tc.tile_pool(name="ps", bufs=6, space="PSUM") as ps,
        ):
            wres = {}
            for g, (ch, cl) in enumerate(groups):
                if ch:
                    t = sb.tile([P, KT, M], bf16, name=f"wg_r{g}")
                    nc.sync.dma_start(t[:], prm[f"wg{g}"][:].rearrange("p (kt m) -> p kt m", m=M))
                    wres[f"wg{g}"] = t
                    t = sb.tile([P, KT, M], bf16, name=f"wu_r{g}")
                    nc.sync.dma_start(t[:], prm[f"wu{g}"][:].rearrange("p (kt m) -> p kt m", m=M))
                    wres[f"wu{g}"] = t
                if cl:
                    t = sb.tile([P, KT, M], f8, name=f"wg8_r{g}")
                    nc.sync.dma_start(t[:], prm[f"wg8_{g}"][:].rearrange("p (kt m) -> p kt m", m=M))
                    wres[f"wg8{g}"] = t
                    t = sb.tile([P, KT, M], f8, name=f"wu8_r{g}")
                    nc.sync.dma_start(t[:], prm[f"wu8_{g}"][:].rearrange("p (kt m) -> p kt m", m=M))
                    wres[f"wu8{g}"] = t
                t = sb.tile([P, MT, H], bf16, name=f"wd_r{g}")
                nc.sync.dma_start(t[:], prm[f"wd{g}"][:].rearrange("p (mt h) -> p mt h", h=H))
                wres[f"wd{g}"] = t

            loop_ctx = ExitStack()
            if loop_n is not None:
                loop_ctx.enter_context(tc.For_i(0, loop_n, 1))

            x_t, x8_t, wc_t, a_t = {}, {}, {}, {}
            for g, (ch, cl) in enumerate(groups):
                cap = ch + cl
                if ch:
                    t = io.tile([P, KT, ch], bf16, name=f"x_t{g}", tag=f"x{g}", bufs=1)
                    nc.sync.dma_start(t[:], prm[f"x{g}"][:].rearrange("p (kt c) -> p kt c", c=ch))
                    x_t[g] = t
                if cl:
                    t = io.tile([P, KT, cl], f8, name=f"x8_t{g}", tag=f"x8{g}", bufs=1)
                    nc.sync.dma_start(t[:], prm[f"x8_{g}"][:].rearrange("p (kt c) -> p kt c", c=cl))
                    x8_t[g] = t
                t = io.tile([P, cap], f32, name=f"wc_t{g}", tag=f"w{g}", bufs=1)
                nc.sync.dma_start(t[:], prm[f"wc{g}"][:])
                wc_t[g] = t
                a_t[g] = io.tile([P, MT, cap], bf16, name=f"a_t{g}", tag=f"a{g}", bufs=1)

            for g, (ch, cl) in enumerate(groups):
                cap = ch + cl
                # bf16 gate/up chains over hi tokens
                for c0, cw in _chunks(ch):
                    for mt in range(MT):
                        psG = ps.tile([P, CHUNK], f32, name=f"psG{g}_{c0}_{mt}", tag="psG", bufs=3)
                        psU = ps.tile([P, CHUNK], f32, name=f"psU{g}_{c0}_{mt}", tag="psU", bufs=3)
                        for kt in range(KT):
                            nc.tensor.matmul(
                                psG[:, :cw],
                                wres[f"wg{g}"][:, kt, mt * P : (mt + 1) * P],
                                x_t[g][:, kt, c0 : c0 + cw],
                                start=(kt == 0), stop=(kt == KT - 1),
                            )
                            nc.tensor.matmul(
                                psU[:, :cw],
                                wres[f"wu{g}"][:, kt, mt * P : (mt + 1) * P],
                                x_t[g][:, kt, c0 : c0 + cw],
                                start=(kt == 0), stop=(kt == KT - 1),
                            )
                        sil = wk.tile([P, CHUNK], f32, name=f"sil{g}_{c0}_{mt}", tag="sil", bufs=2)
                        nc.scalar.activation(sil[:, :cw], psG[:, :cw], AF.Silu)
                        nc.vector.tensor_mul(
                            a_t[g][:, mt, c0 : c0 + cw], sil[:, :cw], psU[:, :cw]
                        )
                # fp8 DoubleRow gate/up chains over lo tokens (a cols offset ch)
                for c0, cw in _chunks(cl):
                    for mt in range(MT):
                        psG = ps.tile([P, CHUNK], f32, name=f"ps8G{g}_{c0}_{mt}", tag="psG", bufs=3)
                        psU = ps.tile([P, CHUNK], f32, name=f"ps8U{g}_{c0}_{mt}", tag="psU", bufs=3)
                        for q in range(KT // 2):
                            nc.tensor.matmul(
                                psG[:, :cw],
                                wres[f"wg8{g}"][:, 2 * q : 2 * q + 2, mt * P : (mt + 1) * P],
                                x8_t[g][:, 2 * q : 2 * q + 2, c0 : c0 + cw],
                                start=(q == 0), stop=(q == KT // 2 - 1),
                                perf_mode=DR,
                            )
                            nc.tensor.matmul(
                                psU[:, :cw],
                                wres[f"wu8{g}"][:, 2 * q : 2 * q + 2, mt * P : (mt + 1) * P],
                                x8_t[g][:, 2 * q : 2 * q + 2, c0 : c0 + cw],
                                start=(q == 0), stop=(q == KT // 2 - 1),
                                perf_mode=DR,
                            )
                        sil = wk.tile([P, CHUNK], f32, name=f"sil8{g}_{c0}_{mt}", tag="sil", bufs=2)
                        nc.scalar.activation(sil[:, :cw], psG[:, :cw], AF.Silu, scale=1.0 / PSCALE)
                        nc.vector.tensor_mul(
                            a_t[g][:, mt, ch + c0 : ch + c0 + cw], sil[:, :cw], psU[:, :cw]
                        )
                # down projection over the combined token range
                for c0, cw in _chunks(cap):
                    for ht in range(HT):
                        psY = ps.tile([P, CHUNK], f32, name=f"psY{g}_{c0}_{ht}", tag="psY", bufs=2)
                        for mt in range(MT):
                            nc.tensor.matmul(
                                psY[:, :cw],
                                wres[f"wd{g}"][:, mt, ht * P : (ht + 1) * P],
                                a_t[g][:, mt, c0 : c0 + cw],
                                start=(mt == 0), stop=(mt == MT - 1),
                            )
                        ysb = wk.tile([P, CHUNK], bf16, name=f"y{g}_{c0}_{ht}", tag="ysb", bufs=4)
                        nc.vector.tensor_mul(
                            ysb[:, :cw], psY[:, :cw], wc_t[g][:, c0 : c0 + cw]
                        )
                        nc.scalar.dma_start(
                            prm[f"y{g}"][:].rearrange("p (ht c) -> p ht c", c=cap)[
                                :, ht, c0 : c0 + cw
                            ],
                            ysb[:, :cw],
                        )

            loop_ctx.close()

    nc.finalize()
    return nc


def _tile_km(w):
    mw = w.shape[1]
    return np.ascontiguousarray(
        w.reshape(KT, P, mw).transpose(1, 0, 2).reshape(P, KT * mw)
    )


def _tile_rhs(w):
    mt = w.shape[0] // P
    return np.ascontiguousarray(
        w.reshape(mt, P, H).transpose(1, 0, 2).reshape(P, mt * H)
    )


def _plan(inputs):
    """Returns (sigs, in_maps, scatter):
    sigs[c] = groups tuple for core c; in_maps[c] = param dict;
    scatter[c] = list of (idx, n) per group (token order hi then lo)."""
    bf = ml_dtypes.bfloat16
    f8 = ml_dtypes.float8_e4m3
    x = np.ascontiguousarray(
        np.asarray(inputs["hidden_states"], dtype=np.float32).reshape(N, H)
    )
    gate_w = np.asarray(inputs["gate_w"], dtype=np.float32)
    Wg = np.asarray(inputs["Wg"], dtype=np.float32)
    Wu = np.asarray(inputs["Wu"], dtype=np.float32)
    Wd = np.asarray(inputs["Wd"], dtype=np.float32)
    sWg = np.asarray(inputs["sWg"], dtype=np.float32)
    sWu = np.asarray(inputs["sWu"], dtype=np.float32)
    sWd = np.asarray(inputs["sWd"], dtype=np.float32)

    logits = x.astype(np.float64) @ gate_w.astype(np.float64).T
    order = np.argsort(-logits, axis=1)
    i1, i2 = order[:, 0], order[:, 1]
    v1 = np.take_along_axis(logits, i1[:, None], 1)[:, 0]
    v2 = np.take_along_axis(logits, i2[:, None], 1)[:, 0]
    ew = np.exp(v2 - v1)
    w1 = 1.0 / (1.0 + ew)
    w2 = ew / (1.0 + ew)

    eidx, ewt = [], []
    for c in range(E):
        sel1, sel2 = i1 == c, i2 == c
        idx = np.nonzero(sel1 | sel2)[0]
        w = np.where(sel1, w1, w2)[idx].astype(np.float32)
        o = np.argsort(-w, kind="stable")
        eidx.append(idx[o])
        ewt.append(w[o])
    counts = np.array([len(ix) for ix in eidx])
    total_w2 = float(w1 @ w1 + w2 @ w2)

    # choose X (bf16 pairs per expert) balancing R vs S core cycles, keeping
    # the fp8 weight-mass fraction under budget
    best = None
    for X in range(120, int(counts.min()) + 1):
        lo = np.sort(counts - X)[::-1]
        L1, L2 = int(lo[0]), int(lo[4])
        cyc_r = (2 * X + 512) * 192
        cyc_s = 512 * 192 + (L1 + L2) * 128
        frac = sum(float(ewt[c][X:] @ ewt[c][X:]) for c in range(E)) / total_w2
        if frac > 0.40:
            continue
        m = max(cyc_r, cyc_s)
        if best is None or m < best[0]:
            best = (m, X, L1, L2, frac)
    if best is None:
        # degenerate routing (extreme imbalance): run every pair in bf16;
        # the fp8 groups become 1-slot dummies computing zeros
        X = int(counts.max())
        best = (0, X, 1, 1, 0.0)
    _, X, L1, L2, frac = best

    lo_counts = counts - X
    lo_order = np.argsort(-lo_counts, kind="stable")

    xT = np.ascontiguousarray(x.T).astype(bf)            # [H, N] bf16
    xT8 = np.ascontiguousarray(x.T * XSCALE).astype(f8)  # [H, N] fp8

    wsets = [(Wg[c], Wu[c], Wd[c]) for c in range(E)]
    wsets.append((sWg[:, :M], sWu[:, :M], sWd[:M, :]))   # shared A
    wsets.append((sWg[:, M:], sWu[:, M:], sWd[M:, :]))   # shared B
    wbf, w8, wdt = {}, {}, {}
    for i, (a, b, d) in enumerate(wsets):
        wdt[i] = _tile_rhs(d.astype(bf))

    def get_bf(i):
        if i not in wbf:
            a, b, d = wsets[i]
            wbf[i] = (_tile_km(a.astype(bf)), _tile_km(b.astype(bf)))
        return wbf[i]

    def get_f8(i):
        if i not in w8:
            a, b, d = wsets[i]
            w8[i] = (
                _tile_km((a * WSCALE).astype(f8)),
                _tile_km((b * WSCALE).astype(f8)),
            )
        return w8[i]

    core_specs = []  # per core: list of (wset, cap_hi, cap_lo, idx, w)
    for i in range(4):  # R cores
        ea, eb = 2 * i, 2 * i + 1
        core_specs.append([
            (ea, X, 0, eidx[ea][:X], ewt[ea][:X]),
            (eb, X, 0, eidx[eb][:X], ewt[eb][:X]),
            (9, 512, 0, np.arange(512 * i, 512 * (i + 1)),
             np.ones(512, np.float32)),
        ])
    for i in range(4):  # S cores
        ca, cb = int(lo_order[i]), int(lo_order[7 - i])
        core_specs.append([
            (8, 512, 0, np.arange(512 * i, 512 * (i + 1)),
             np.ones(512, np.float32)),
            (ca, 0, L1, eidx[ca][X:], ewt[ca][X:]),
            (cb, 0, L2, eidx[cb][X:], ewt[cb][X:]),
        ])

    sigs, in_maps, scatter = [], [], []
    for spec in core_specs:
        sig, m, sc = [], {}, []
        for g, (ws, ch, cl, idx, w) in enumerate(spec):
            n = len(idx)
            cap = ch + cl
            assert n <= cap, (n, cap)
            sig.append((ch, cl))
            w_pad = np.zeros(cap, np.float32)
            if ch:
                xe = np.zeros((H, ch), dtype=bf)
                xe[:, :n] = xT[:, idx]
                m[f"x{g}"] = _tile_km(xe)
                tg, tu = get_bf(ws)
                m[f"wg{g}"] = tg
                m[f"wu{g}"] = tu
                w_pad[:n] = w
            else:
                xe = np.zeros((H, cl), dtype=f8)
                xe[:, :n] = xT8[:, idx]
                m[f"x8_{g}"] = _tile_km(xe)
                tg, tu = get_f8(ws)
                m[f"wg8_{g}"] = tg
                m[f"wu8_{g}"] = tu
                w_pad[:n] = w / PSCALE
            m[f"wd{g}"] = wdt[ws]
            m[f"wc{g}"] = np.ascontiguousarray(
                np.broadcast_to(w_pad[None, :], (P, cap))
            )
            sc.append((idx, n))
        sigs.append(tuple(sig))
        in_maps.append(m)
        scatter.append(sc)
    return sigs, in_maps, scatter


def _prep_in_maps(inputs) -> list:
    return _plan(inputs)[1]


def _unshard(results, sigs, scatter) -> np.ndarray:
    y = np.zeros((N, H), np.float32)
    for c in range(NCORES):
        for g, (idx, n) in enumerate(scatter[c]):
            ch, cl = sigs[c][g]
            cap = ch + cl
            arr = results[c][f"y{g}"].reshape(P, HT, cap)[:, :, :n]
            y[idx] += arr.transpose(2, 1, 0).reshape(n, H).astype(np.float32)
    return y.reshape(B, S, H)


def kernel(**inputs) -> np.ndarray:
    from concourse.bass_utils import run_bass_kernel_spmd

    sigs, in_maps, scatter = _plan(inputs)
    results = [None] * NCORES
    done = set()
    for c in range(NCORES):
        if c in done:
            continue
        cores = [d for d in range(NCORES) if sigs[d] == sigs[c]]
        key = ("v3", sigs[c])
        if key not in _CACHE:
            _CACHE[key] = _build_program(groups=sigs[c])
        res = run_bass_kernel_spmd(
            _CACHE[key], [in_maps[d] for d in cores], list(range(len(cores)))
        ).results
        for j, d in enumerate(cores):
            results[d] = res[j]
            done.add(d)
    return _unshard(results, sigs, scatter)


if __name__ == "__main__":
    sys.path.insert(0, "/root/problem")
    import reference

    inp = reference.setup_inputs()
    expected = np.asarray(reference.reference(**inp))
    actual = kernel(**{k: np.asarray(v) for k, v in inp.items()})
    err = np.linalg.norm(actual - expected) / np.linalg.norm(expected)
    print("Relative error:", err)


# revision 15
# speedup vs baseline: 1.0380x; 1.0380x over previous
"""MoE layer (8 routed experts, top-2, shared experts) on 8 Trainium2 cores,
with selective fp8 (e4m3 DoubleRow) for low-combine-weight routed pairs.

Work layout: the shared expert is split into two virtual experts A/B of
intermediate 512 (combine weight 1), giving 8192 equal-cost token-expert
pairs. Each routed pair's gate/up can run in fp8 at 2x PE throughput; the
quantization error budget allows this only for pairs with small combine
weight, so each expert's pairs are split into its top-X by weight ("hi",
bf16) and the rest ("lo", fp8 gate/up, bf16 down).

Cores 0-3 ("R"): two expert-hi groups of exactly X tokens + a 512-token
sharedB chunk — all bf16. Cores 4-7 ("S"): a 512-token sharedA chunk (bf16)
+ two expert-lo fp8 groups. X is chosen so both core types have equal cycle
counts; shared chunks tile A/B exactly, expert-hi groups are exact by
construction, so padding is limited to the fp8 group capacities.

Matmuls are weights-stationary: gate/up produce [m=128, tokens] in PSUM
(tokens on the free dim — group sizes need no 128-padding), SwiGLU applies
in that layout, and down consumes it directly as stationary — no PE
transposes. fp8 scaling: x8 = e4m3(8x), w8 = e4m3(64W), so PSUM holds 512*g;
Silu rescales by 1/512 and the remaining 512 on the up path is folded into
the per-token combine weight (w/512) applied at the PSUM drain. The Act
engine only runs Silu (no activation-table swaps).
"""

import sys

if "/opt/trn_rl_repo" not in sys.path:
    sys.path.insert(0, "/opt/trn_rl_repo")

import ml_dtypes
import numpy as np

B, S, H = 2, 1024, 2048
N = B * S
E = 8
M = 512
P = 128
KT = H // P
MT = M // P
HT = H // P
NCORES = 8
CHUNK = 512
XSCALE = 8.0      # fp8 activation pre-scale
WSCALE = 64.0     # fp8 weight pre-scale
PSCALE = XSCALE * WSCALE

_CACHE = {}


def _chunks(n):
    if n == 0:
        return []
    k = -(-n // CHUNK)
    base, rem = divmod(n, k)
    out, c0 = [], 0
    for i in range(k):
        w = base + (1 if i < rem else 0)
        out.append((c0, w))
        c0 += w
    return out


def _build_program(collectives=True, loop_n=None, groups=((205, 0), (205, 0), (512, 0))):
    """groups: tuple of (cap_hi, cap_lo) per group."""
    import concourse.mybir as mybir
    import concourse.tile as tile
    from concourse import bacc
    from contextlib import ExitStack

    f32 = mybir.dt.float32
    bf16 = mybir.dt.bfloat16
    f8 = mybir.dt.float8e4
    AF = mybir.ActivationFunctionType
    DR = mybir.MatmulPerfMode.DoubleRow

    nc = bacc.Bacc(None)
    G = len(groups)

    prm = {}
    for g, (ch, cl) in enumerate(groups):
        cap = ch + cl
        if ch:
            prm[f"x{g}"] = nc.declare_dram_parameter(f"x{g}", [P, KT * ch], bf16, isOutput=False)
            prm[f"wg{g}"] = nc.declare_dram_parameter(f"wg{g}", [P, KT * M], bf16, isOutput=False)
            prm[f"wu{g}"] = nc.declare_dram_parameter(f"wu{g}", [P, KT * M], bf16, isOutput=False)
        if cl:
            prm[f"x8_{g}"] = nc.declare_dram_parameter(f"x8_{g}", [P, KT * cl], f8, isOutput=False)
            prm[f"wg8_{g}"] = nc.declare_dram_parameter(f"wg8_{g}", [P, KT * M], f8, isOutput=False)
            prm[f"wu8_{g}"] = nc.declare_dram_parameter(f"wu8_{g}", [P, KT * M], f8, isOutput=False)
        if ch:
            prm[f"wd{g}"] = nc.declare_dram_parameter(f"wd{g}", [P, MT * H], bf16, isOutput=False)
        if cl:
            prm[f"wd8_{g}"] = nc.declare_dram_parameter(f"wd8_{g}", [P, MT * H], f8, isOutput=False)
        prm[f"wc{g}"] = nc.declare_dram_parameter(f"wc{g}", [P, cap], f32, isOutput=False)
        prm[f"y{g}"] = nc.declare_dram_parameter(f"y{g}", [P, HT * cap], bf16, isOutput=True)

    with tile.TileContext(nc) as tc:
        with (
            tc.tile_pool(name="sb", bufs=1) as sb,
            tc.tile_pool(name="io", bufs=2) as io,
            tc.tile_pool(name="wk", bufs=2) as wk,
            tc.tile_pool(name="ps", bufs=6, space="PSUM") as ps,
        ):
            wres = {}
            for g, (ch, cl) in enumerate(groups):
                if ch:
                    t = sb.tile([P, KT, M], bf16, name=f"wg_r{g}")
                    nc.sync.dma_start(t[:], prm[f"wg{g}"][:].rearrange("p (kt m) -> p kt m", m=M))
                    wres[f"wg{g}"] = t
                    t = sb.tile([P, KT, M], bf16, name=f"wu_r{g}")
                    nc.sync.dma_start(t[:], prm[f"wu{g}"][:].rearrange("p (kt m) -> p kt m", m=M))
                    wres[f"wu{g}"] = t
                if cl:
                    t = sb.tile([P, KT, M], f8, name=f"wg8_r{g}")
                    nc.sync.dma_start(t[:], prm[f"wg8_{g}"][:].rearrange("p (kt m) -> p kt m", m=M))
                    wres[f"wg8{g}"] = t
                    t = sb.tile([P, KT, M], f8, name=f"wu8_r{g}")
                    nc.sync.dma_start(t[:], prm[f"wu8_{g}"][:].rearrange("p (kt m) -> p kt m", m=M))
                    wres[f"wu8{g}"] = t
                if ch:
                    t = sb.tile([P, MT, H], bf16, name=f"wd_r{g}")
                    nc.sync.dma_start(t[:], prm[f"wd{g}"][:].rearrange("p (mt h) -> p mt h", h=H))
                    wres[f"wd{g}"] = t
                if cl:
                    t = sb.tile([P, MT, H], f8, name=f"wd8_r{g}")
                    nc.sync.dma_start(t[:], prm[f"wd8_{g}"][:].rearrange("p (mt h) -> p mt h", h=H))
                    wres[f"wd8{g}"] = t

            loop_ctx = ExitStack()
            if loop_n is not None:
                loop_ctx.enter_context(tc.For_i(0, loop_n, 1))

            x_t, x8_t, wc_t, a_t = {}, {}, {}, {}
            for g, (ch, cl) in enumerate(groups):
                cap = ch + cl
                if ch:
                    t = io.tile([P, KT, ch], bf16, name=f"x_t{g}", tag=f"x{g}", bufs=1)
                    nc.sync.dma_start(t[:], prm[f"x{g}"][:].rearrange("p (kt c) -> p kt c", c=ch))
                    x_t[g] = t
                if cl:
                    t = io.tile([P, KT, cl], f8, name=f"x8_t{g}", tag=f"x8{g}", bufs=1)
                    nc.sync.dma_start(t[:], prm[f"x8_{g}"][:].rearrange("p (kt c) -> p kt c", c=cl))
                    x8_t[g] = t
                t = io.tile([P, cap], f32, name=f"wc_t{g}", tag=f"w{g}", bufs=1)
                nc.sync.dma_start(t[:], prm[f"wc{g}"][:])
                wc_t[g] = t
                if ch:
                    a_t[g] = io.tile([P, MT, ch], bf16, name=f"a_t{g}", tag=f"a{g}", bufs=1)
                if cl:
                    x8_t[f"a8{g}"] = io.tile([P, MT, cl], f8, name=f"a8_t{g}", tag=f"a8{g}", bufs=1)

            for g, (ch, cl) in enumerate(groups):
                cap = ch + cl
                # bf16 gate/up chains over hi tokens
                for c0, cw in _chunks(ch):
                    for mt in range(MT):
                        psG = ps.tile([P, CHUNK], f32, name=f"psG{g}_{c0}_{mt}", tag="psG", bufs=3)
                        psU = ps.tile([P, CHUNK], f32, name=f"psU{g}_{c0}_{mt}", tag="psU", bufs=3)
                        for kt in range(KT):
                            nc.tensor.matmul(
                                psG[:, :cw],
                                wres[f"wg{g}"][:, kt, mt * P : (mt + 1) * P],
                                x_t[g][:, kt, c0 : c0 + cw],
                                start=(kt == 0), stop=(kt == KT - 1),
                            )
                            nc.tensor.matmul(
                                psU[:, :cw],
                                wres[f"wu{g}"][:, kt, mt * P : (mt + 1) * P],
                                x_t[g][:, kt, c0 : c0 + cw],
                                start=(kt == 0), stop=(kt == KT - 1),
                            )
                        sil = wk.tile([P, CHUNK], f32, name=f"sil{g}_{c0}_{mt}", tag="sil", bufs=2)
                        nc.scalar.activation(sil[:, :cw], psG[:, :cw], AF.Silu)
                        nc.vector.tensor_mul(
                            a_t[g][:, mt, c0 : c0 + cw], sil[:, :cw], psU[:, :cw]
                        )
                # fp8 DoubleRow gate/up chains over lo tokens (a cols offset ch)
                for c0, cw in _chunks(cl):
                    for mt in range(MT):
                        psG = ps.tile([P, CHUNK], f32, name=f"ps8G{g}_{c0}_{mt}", tag="psG", bufs=3)
                        psU = ps.tile([P, CHUNK], f32, name=f"ps8U{g}_{c0}_{mt}", tag="psU", bufs=3)
                        for q in range(KT // 2):
                            nc.tensor.matmul(
                                psG[:, :cw],
                                wres[f"wg8{g}"][:, 2 * q : 2 * q + 2, mt * P : (mt + 1) * P],
                                x8_t[g][:, 2 * q : 2 * q + 2, c0 : c0 + cw],
                                start=(q == 0), stop=(q == KT // 2 - 1),
                                perf_mode=DR,
                            )
                            nc.tensor.matmul(
                                psU[:, :cw],
                                wres[f"wu8{g}"][:, 2 * q : 2 * q + 2, mt * P : (mt + 1) * P],
                                x8_t[g][:, 2 * q : 2 * q + 2, c0 : c0 + cw],
                                start=(q == 0), stop=(q == KT // 2 - 1),
                                perf_mode=DR,
                            )
                        sil = wk.tile([P, CHUNK], f32, name=f"sil8{g}_{c0}_{mt}", tag="sil", bufs=2)
                        nc.scalar.activation(sil[:, :cw], psG[:, :cw], AF.Silu, scale=1.0 / PSCALE)
                        nc.vector.scalar_tensor_tensor(
                            x8_t[f"a8{g}"][:, mt, c0 : c0 + cw],
                            psU[:, :cw], 1.0 / WSCALE, sil[:, :cw],
                            mybir.AluOpType.mult, mybir.AluOpType.mult,
                        )
                # down projection over the combined token range
                for c0, cw in _chunks(ch):
                    for ht in range(HT):
                        psY = ps.tile([P, CHUNK], f32, name=f"psY{g}_{c0}_{ht}", tag="psY", bufs=2)
                        for mt in range(MT):
                            nc.tensor.matmul(
                                psY[:, :cw],
                                wres[f"wd{g}"][:, mt, ht * P : (ht + 1) * P],
                                a_t[g][:, mt, c0 : c0 + cw],
                                start=(mt == 0), stop=(mt == MT - 1),
                            )
                        ysb = wk.tile([P, CHUNK], bf16, name=f"y{g}_{c0}_{ht}", tag="ysb", bufs=4)
                        nc.vector.tensor_mul(
                            ysb[:, :cw], psY[:, :cw], wc_t[g][:, c0 : c0 + cw]
                        )
                        nc.scalar.dma_start(
                            prm[f"y{g}"][:].rearrange("p (ht c) -> p ht c", c=cap)[
                                :, ht, c0 : c0 + cw
                            ],
                            ysb[:, :cw],
                        )
                for c0, cw in _chunks(cl):
                    for ht in range(HT):
                        psY = ps.tile([P, CHUNK], f32, name=f"psY8{g}_{c0}_{ht}", tag="psY", bufs=2)
                        for q in range(MT // 2):
                            nc.tensor.matmul(
                                psY[:, :cw],
                                wres[f"wd8{g}"][:, 2 * q : 2 * q + 2, ht * P : (ht + 1) * P],
                                x8_t[f"a8{g}"][:, 2 * q : 2 * q + 2, c0 : c0 + cw],
                                start=(q == 0), stop=(q == MT // 2 - 1),
                                perf_mode=DR,
                            )
                        ysb = wk.tile([P, CHUNK], bf16, name=f"y8{g}_{c0}_{ht}", tag="ysb", bufs=4)
                        nc.vector.tensor_mul(
                            ysb[:, :cw], psY[:, :cw], wc_t[g][:, ch + c0 : ch + c0 + cw]
                        )
                        nc.scalar.dma_start(
                            prm[f"y{g}"][:].rearrange("p (ht c) -> p ht c", c=cap)[
                                :, ht, ch + c0 : ch + c0 + cw
                            ],
                            ysb[:, :cw],
                        )

            loop_ctx.close()

    nc.finalize()
    return nc


def _tile_km(w):
    mw = w.shape[1]
    return np.ascontiguousarray(
        w.reshape(KT, P, mw).transpose(1, 0, 2).reshape(P, KT * mw)
    )


def _tile_rhs(w):
    mt = w.shape[0] // P
    return np.ascontiguousarray(
        w.reshape(mt, P, H).transpose(1, 0, 2).reshape(P, mt * H)
    )


def _plan(inputs):
    """Returns (sigs, in_maps, scatter):
    sigs[c] = groups tuple for core c; in_maps[c] = param dict;
    scatter[c] = list of (idx, n) per group (token order hi then lo)."""
    bf = ml_dtypes.bfloat16
    f8 = ml_dtypes.float8_e4m3
    x = np.ascontiguousarray(
        np.asarray(inputs["hidden_states"], dtype=np.float32).reshape(N, H)
    )
    gate_w = np.asarray(inputs["gate_w"], dtype=np.float32)
    Wg = np.asarray(inputs["Wg"], dtype=np.float32)
    Wu = np.asarray(inputs["Wu"], dtype=np.float32)
    Wd = np.asarray(inputs["Wd"], dtype=np.float32)
    sWg = np.asarray(inputs["sWg"], dtype=np.float32)
    sWu = np.asarray(inputs["sWu"], dtype=np.float32)
    sWd = np.asarray(inputs["sWd"], dtype=np.float32)

    logits = x.astype(np.float64) @ gate_w.astype(np.float64).T
    order = np.argsort(-logits, axis=1)
    i1, i2 = order[:, 0], order[:, 1]
    v1 = np.take_along_axis(logits, i1[:, None], 1)[:, 0]
    v2 = np.take_along_axis(logits, i2[:, None], 1)[:, 0]
    ew = np.exp(v2 - v1)
    w1 = 1.0 / (1.0 + ew)
    w2 = ew / (1.0 + ew)

    eidx, ewt = [], []
    for c in range(E):
        sel1, sel2 = i1 == c, i2 == c
        idx = np.nonzero(sel1 | sel2)[0]
        w = np.where(sel1, w1, w2)[idx].astype(np.float32)
        o = np.argsort(-w, kind="stable")
        eidx.append(idx[o])
        ewt.append(w[o])
    counts = np.array([len(ix) for ix in eidx])
    total_w2 = float(w1 @ w1 + w2 @ w2)

    # choose X (bf16 pairs per expert) balancing R vs S core cycles, keeping
    # the fp8 weight-mass fraction under budget
    best = None
    for X in range(120, int(counts.min()) + 1):
        lo = np.sort(counts - X)[::-1]
        L1, L2 = int(lo[0]), int(lo[4])
        cyc_r = (2 * X + 512) * 192
        cyc_s = 512 * 192 + (L1 + L2) * 96
        frac = sum(float(ewt[c][X:] @ ewt[c][X:]) for c in range(E)) / total_w2
        if frac > 0.37:
            continue
        m = max(cyc_r, cyc_s)
        if best is None or m < best[0]:
            best = (m, X, L1, L2, frac)
    if best is None:
        # degenerate routing (extreme imbalance): run every pair in bf16;
        # the fp8 groups become 1-slot dummies computing zeros
        X = int(counts.max())
        best = (0, X, 1, 1, 0.0)
    _, X, L1, L2, frac = best

    lo_counts = counts - X
    lo_order = np.argsort(-lo_counts, kind="stable")

    xT = np.ascontiguousarray(x.T).astype(bf)            # [H, N] bf16
    xT8 = np.ascontiguousarray(x.T * XSCALE).astype(f8)  # [H, N] fp8

    wsets = [(Wg[c], Wu[c], Wd[c]) for c in range(E)]
    wsets.append((sWg[:, :M], sWu[:, :M], sWd[:M, :]))   # shared A
    wsets.append((sWg[:, M:], sWu[:, M:], sWd[M:, :]))   # shared B
    wbf, w8, wdt = {}, {}, {}
    for i, (a, b, d) in enumerate(wsets):
        wdt[i] = _tile_rhs(d.astype(bf))

    def get_bf(i):
        if i not in wbf:
            a, b, d = wsets[i]
            wbf[i] = (_tile_km(a.astype(bf)), _tile_km(b.astype(bf)))
        return wbf[i]

    w8d = {}

    def get_f8d(i):
        if i not in w8d:
            a, b, d = wsets[i]
            w8d[i] = _tile_rhs((d * WSCALE).astype(f8))
        return w8d[i]

    def get_f8(i):
        if i not in w8:
            a, b, d = wsets[i]
            w8[i] = (
                _tile_km((a * WSCALE).astype(f8)),
                _tile_km((b * WSCALE).astype(f8)),
            )
        return w8[i]

    core_specs = []  # per core: list of (wset, cap_hi, cap_lo, idx, w)
    for i in range(4):  # R cores
        ea, eb = 2 * i, 2 * i + 1
        core_specs.append([
            (ea, X, 0, eidx[ea][:X], ewt[ea][:X]),
            (eb, X, 0, eidx[eb][:X], ewt[eb][:X]),
            (9, 512, 0, np.arange(512 * i, 512 * (i + 1)),
             np.ones(512, np.float32)),
        ])
    for i in range(4):  # S cores
        ca, cb = int(lo_order[i]), int(lo_order[7 - i])
        core_specs.append([
            (8, 512, 0, np.arange(512 * i, 512 * (i + 1)),
             np.ones(512, np.float32)),
            (ca, 0, L1, eidx[ca][X:], ewt[ca][X:]),
            (cb, 0, L2, eidx[cb][X:], ewt[cb][X:]),
        ])

    sigs, in_maps, scatter = [], [], []
    for spec in core_specs:
        sig, m, sc = [], {}, []
        for g, (ws, ch, cl, idx, w) in enumerate(spec):
            n = len(idx)
            cap = ch + cl
            assert n <= cap, (n, cap)
            sig.append((ch, cl))
            w_pad = np.zeros(cap, np.float32)
            if ch:
                xe = np.zeros((H, ch), dtype=bf)
                xe[:, :n] = xT[:, idx]
                m[f"x{g}"] = _tile_km(xe)
                tg, tu = get_bf(ws)
                m[f"wg{g}"] = tg
                m[f"wu{g}"] = tu
                w_pad[:n] = w
            else:
                xe = np.zeros((H, cl), dtype=f8)
                xe[:, :n] = xT8[:, idx]
                m[f"x8_{g}"] = _tile_km(xe)
                tg, tu = get_f8(ws)
                m[f"wg8_{g}"] = tg
                m[f"wu8_{g}"] = tu
                w_pad[:n] = w / PSCALE
            if ch:
                m[f"wd{g}"] = wdt[ws]
            if cl:
                m[f"wd8_{g}"] = get_f8d(ws)
            m[f"wc{g}"] = np.ascontiguousarray(
                np.broadcast_to(w_pad[None, :], (P, cap))
            )
            sc.append((idx, n))
        sigs.append(tuple(sig))
        in_maps.append(m)
        scatter.append(sc)
    return sigs, in_maps, scatter


def _prep_in_maps(inputs) -> list:
    return _plan(inputs)[1]


def _unshard(results, sigs, scatter) -> np.ndarray:
    y = np.zeros((N, H), np.float32)
    for c in range(NCORES):
        for g, (idx, n) in enumerate(scatter[c]):
            ch, cl = sigs[c][g]
            cap = ch + cl
            arr = results[c][f"y{g}"].reshape(P, HT, cap)[:, :, :n]
            y[idx] += arr.transpose(2, 1, 0).reshape(n, H).astype(np.float32)
    return y.reshape(B, S, H)


def kernel(**inputs) -> np.ndarray:
    from concourse.bass_utils import run_bass_kernel_spmd

    sigs, in_maps, scatter = _plan(inputs)
    results = [None] * NCORES
    done = set()
    for c in range(NCORES):
        if c in done:
            continue
        cores = [d for d in range(NCORES) if sigs[d] == sigs[c]]
        key = ("v3", sigs[c])
        if key not in _CACHE:
            _CACHE[key] = _build_program(groups=sigs[c])
        res = run_bass_kernel_spmd(
            _CACHE[key], [in_maps[d] for d in cores], list(range(len(cores)))
        ).results
        for j, d in enumerate(cores):
            results[d] = res[j]
            done.add(d)
    return _unshard(results, sigs, scatter)


if __name__ == "__main__":
    sys.path.insert(0, "/root/problem")
    import reference

    inp = reference.setup_inputs()
    expected = np.asarray(reference.reference(**inp))
    actual = kernel(**{k: np.asarray(v) for k, v in inp.items()})
    err = np.linalg.norm(actual - expected) / np.linalg.norm(expected)
    print("Relative error:", err)


# revision 16
# speedup vs baseline: 1.0569x; 1.0183x over previous
"""MoE layer (8 routed experts, top-2, shared experts) on 8 Trainium2 cores,
with selective fp8 (e4m3 DoubleRow) for low-combine-weight routed pairs.

Work layout: the shared expert is split into two virtual experts A/B of
intermediate 512 (combine weight 1), giving 8192 equal-cost token-expert
pairs. Each routed pair's gate/up can run in fp8 at 2x PE throughput; the
quantization error budget allows this only for pairs with small combine
weight, so each expert's pairs are split into its top-X by weight ("hi",
bf16) and the rest ("lo", fp8 gate/up, bf16 down).

Cores 0-3 ("R"): two expert-hi groups of exactly X tokens + a 512-token
sharedB chunk — all bf16. Cores 4-7 ("S"): a 512-token sharedA chunk (bf16)
+ two expert-lo fp8 groups. X is chosen so both core types have equal cycle
counts; shared chunks tile A/B exactly, expert-hi groups are exact by
construction, so padding is limited to the fp8 group capacities.

Matmuls are weights-stationary: gate/up produce [m=128, tokens] in PSUM
(tokens on the free dim — group sizes need no 128-padding), SwiGLU applies
in that layout, and down consumes it directly as stationary — no PE
transposes. fp8 scaling: x8 = e4m3(8x), w8 = e4m3(64W), so PSUM holds 512*g;
Silu rescales by 1/512 and the remaining 512 on the up path is folded into
the per-token combine weight (w/512) applied at the PSUM drain. The Act
engine only runs Silu (no activation-table swaps).
"""

import sys

if "/opt/trn_rl_repo" not in sys.path:
    sys.path.insert(0, "/opt/trn_rl_repo")

import ml_dtypes
import numpy as np

B, S, H = 2, 1024, 2048
N = B * S
E = 8
M = 512
P = 128
KT = H // P
MT = M // P
HT = H // P
NCORES = 8
CHUNK = 512
XSCALE = 8.0      # fp8 activation pre-scale
WSCALE = 64.0     # fp8 weight pre-scale
PSCALE = XSCALE * WSCALE

_CACHE = {}


def _chunks(n):
    if n == 0:
        return []
    k = -(-n // CHUNK)
    base, rem = divmod(n, k)
    out, c0 = [], 0
    for i in range(k):
        w = base + (1 if i < rem else 0)
        out.append((c0, w))
        c0 += w
    return out


def _build_program(collectives=True, loop_n=None, groups=((205, 0), (205, 0), (512, 0))):
    """groups: tuple of (cap_hi, cap_lo) per group."""
    import concourse.mybir as mybir
    import concourse.tile as tile
    from concourse import bacc
    from contextlib import ExitStack

    f32 = mybir.dt.float32
    bf16 = mybir.dt.bfloat16
    f8 = mybir.dt.float8e4
    AF = mybir.ActivationFunctionType
    DR = mybir.MatmulPerfMode.DoubleRow

    nc = bacc.Bacc(None)
    G = len(groups)

    prm = {}
    for g, (ch, cl) in enumerate(groups):
        cap = ch + cl
        if ch:
            prm[f"x{g}"] = nc.declare_dram_parameter(f"x{g}", [P, KT * ch], bf16, isOutput=False)
            prm[f"wg{g}"] = nc.declare_dram_parameter(f"wg{g}", [P, KT * M], bf16, isOutput=False)
            prm[f"wu{g}"] = nc.declare_dram_parameter(f"wu{g}", [P, KT * M], bf16, isOutput=False)
        if cl:
            prm[f"x8_{g}"] = nc.declare_dram_parameter(f"x8_{g}", [P, KT * cl], f8, isOutput=False)
            prm[f"wg8_{g}"] = nc.declare_dram_parameter(f"wg8_{g}", [P, KT * M], f8, isOutput=False)
            prm[f"wu8_{g}"] = nc.declare_dram_parameter(f"wu8_{g}", [P, KT * M], f8, isOutput=False)
        if ch:
            prm[f"wd{g}"] = nc.declare_dram_parameter(f"wd{g}", [P, MT * H], bf16, isOutput=False)
        if cl:
            prm[f"wd8_{g}"] = nc.declare_dram_parameter(f"wd8_{g}", [P, MT * H], f8, isOutput=False)
        prm[f"wc{g}"] = nc.declare_dram_parameter(f"wc{g}", [P, cap], f32, isOutput=False)
        prm[f"y{g}"] = nc.declare_dram_parameter(f"y{g}", [P, HT * cap], bf16, isOutput=True)

    with tile.TileContext(nc) as tc:
        with (
            tc.tile_pool(name="sb", bufs=1) as sb,
            tc.tile_pool(name="io", bufs=2) as io,
            tc.tile_pool(name="wk", bufs=2) as wk,
            tc.tile_pool(name="ps", bufs=6, space="PSUM") as ps,
        ):
            wres = {}
            for g, (ch, cl) in enumerate(groups):
                if ch:
                    t = sb.tile([P, KT, M], bf16, name=f"wg_r{g}")
                    nc.sync.dma_start(t[:], prm[f"wg{g}"][:].rearrange("p (kt m) -> p kt m", m=M))
                    wres[f"wg{g}"] = t
                    t = sb.tile([P, KT, M], bf16, name=f"wu_r{g}")
                    nc.sync.dma_start(t[:], prm[f"wu{g}"][:].rearrange("p (kt m) -> p kt m", m=M))
                    wres[f"wu{g}"] = t
                if cl:
                    t = sb.tile([P, KT, M], f8, name=f"wg8_r{g}")
                    nc.sync.dma_start(t[:], prm[f"wg8_{g}"][:].rearrange("p (kt m) -> p kt m", m=M))
                    wres[f"wg8{g}"] = t
                    t = sb.tile([P, KT, M], f8, name=f"wu8_r{g}")
                    nc.sync.dma_start(t[:], prm[f"wu8_{g}"][:].rearrange("p (kt m) -> p kt m", m=M))
                    wres[f"wu8{g}"] = t
                if ch:
                    t = sb.tile([P, MT, H], bf16, name=f"wd_r{g}")
                    nc.sync.dma_start(t[:], prm[f"wd{g}"][:].rearrange("p (mt h) -> p mt h", h=H))
                    wres[f"wd{g}"] = t
                if cl:
                    t = sb.tile([P, MT, H], f8, name=f"wd8_r{g}")
                    nc.sync.dma_start(t[:], prm[f"wd8_{g}"][:].rearrange("p (mt h) -> p mt h", h=H))
                    wres[f"wd8{g}"] = t

            loop_ctx = ExitStack()
            if loop_n is not None:
                loop_ctx.enter_context(tc.For_i(0, loop_n, 1))

            x_t, x8_t, wc_t, a_t = {}, {}, {}, {}
            for g, (ch, cl) in enumerate(groups):
                cap = ch + cl
                if ch:
                    t = io.tile([P, KT, ch], bf16, name=f"x_t{g}", tag=f"x{g}", bufs=1)
                    nc.sync.dma_start(t[:], prm[f"x{g}"][:].rearrange("p (kt c) -> p kt c", c=ch))
                    x_t[g] = t
                if cl:
                    t = io.tile([P, KT, cl], f8, name=f"x8_t{g}", tag=f"x8{g}", bufs=1)
                    nc.sync.dma_start(t[:], prm[f"x8_{g}"][:].rearrange("p (kt c) -> p kt c", c=cl))
                    x8_t[g] = t
                t = io.tile([P, cap], f32, name=f"wc_t{g}", tag=f"w{g}", bufs=1)
                nc.sync.dma_start(t[:], prm[f"wc{g}"][:])
                wc_t[g] = t
                if ch:
                    a_t[g] = io.tile([P, MT, ch], bf16, name=f"a_t{g}", tag=f"a{g}", bufs=1)
                if cl:
                    x8_t[f"a8{g}"] = io.tile([P, MT, cl], f8, name=f"a8_t{g}", tag=f"a8{g}", bufs=1)

            for g, (ch, cl) in enumerate(groups):
                cap = ch + cl
                # bf16 gate/up chains over hi tokens
                for c0, cw in _chunks(ch):
                    for mt in range(MT):
                        psG = ps.tile([P, CHUNK], f32, name=f"psG{g}_{c0}_{mt}", tag="psG", bufs=3)
                        psU = ps.tile([P, CHUNK], f32, name=f"psU{g}_{c0}_{mt}", tag="psU", bufs=3)
                        for kt in range(KT):
                            nc.tensor.matmul(
                                psG[:, :cw],
                                wres[f"wg{g}"][:, kt, mt * P : (mt + 1) * P],
                                x_t[g][:, kt, c0 : c0 + cw],
                                start=(kt == 0), stop=(kt == KT - 1),
                            )
                            nc.tensor.matmul(
                                psU[:, :cw],
                                wres[f"wu{g}"][:, kt, mt * P : (mt + 1) * P],
                                x_t[g][:, kt, c0 : c0 + cw],
                                start=(kt == 0), stop=(kt == KT - 1),
                            )
                        sil = wk.tile([P, CHUNK], f32, name=f"sil{g}_{c0}_{mt}", tag="sil", bufs=2)
                        nc.scalar.activation(sil[:, :cw], psG[:, :cw], AF.Silu)
                        nc.vector.tensor_mul(
                            a_t[g][:, mt, c0 : c0 + cw], sil[:, :cw], psU[:, :cw]
                        )
                # fp8 DoubleRow gate/up chains over lo tokens (a cols offset ch)
                for c0, cw in _chunks(cl):
                    for mt in range(MT):
                        psG = ps.tile([P, CHUNK], f32, name=f"ps8G{g}_{c0}_{mt}", tag="psG", bufs=3)
                        psU = ps.tile([P, CHUNK], f32, name=f"ps8U{g}_{c0}_{mt}", tag="psU", bufs=3)
                        for q in range(KT // 2):
                            nc.tensor.matmul(
                                psG[:, :cw],
                                wres[f"wg8{g}"][:, 2 * q : 2 * q + 2, mt * P : (mt + 1) * P],
                                x8_t[g][:, 2 * q : 2 * q + 2, c0 : c0 + cw],
                                start=(q == 0), stop=(q == KT // 2 - 1),
                                perf_mode=DR,
                            )
                            nc.tensor.matmul(
                                psU[:, :cw],
                                wres[f"wu8{g}"][:, 2 * q : 2 * q + 2, mt * P : (mt + 1) * P],
                                x8_t[g][:, 2 * q : 2 * q + 2, c0 : c0 + cw],
                                start=(q == 0), stop=(q == KT // 2 - 1),
                                perf_mode=DR,
                            )
                        sil = wk.tile([P, CHUNK], f32, name=f"sil8{g}_{c0}_{mt}", tag="sil", bufs=2)
                        nc.scalar.activation(sil[:, :cw], psG[:, :cw], AF.Silu, scale=1.0 / PSCALE)
                        nc.vector.scalar_tensor_tensor(
                            x8_t[f"a8{g}"][:, mt, c0 : c0 + cw],
                            psU[:, :cw], 1.0 / WSCALE, sil[:, :cw],
                            mybir.AluOpType.mult, mybir.AluOpType.mult,
                        )
                # down projection over the combined token range
                for c0, cw in _chunks(ch):
                    for ht in range(HT):
                        psY = ps.tile([P, CHUNK], f32, name=f"psY{g}_{c0}_{ht}", tag="psY", bufs=2)
                        for mt in range(MT):
                            nc.tensor.matmul(
                                psY[:, :cw],
                                wres[f"wd{g}"][:, mt, ht * P : (ht + 1) * P],
                                a_t[g][:, mt, c0 : c0 + cw],
                                start=(mt == 0), stop=(mt == MT - 1),
                            )
                        ysb = wk.tile([P, CHUNK], bf16, name=f"y{g}_{c0}_{ht}", tag="ysb", bufs=4)
                        nc.vector.tensor_mul(
                            ysb[:, :cw], psY[:, :cw], wc_t[g][:, c0 : c0 + cw]
                        )
                        nc.scalar.dma_start(
                            prm[f"y{g}"][:].rearrange("p (ht c) -> p ht c", c=cap)[
                                :, ht, c0 : c0 + cw
                            ],
                            ysb[:, :cw],
                        )
                for c0, cw in _chunks(cl):
                    for ht in range(HT):
                        psY = ps.tile([P, CHUNK], f32, name=f"psY8{g}_{c0}_{ht}", tag="psY", bufs=2)
                        for q in range(MT // 2):
                            nc.tensor.matmul(
                                psY[:, :cw],
                                wres[f"wd8{g}"][:, 2 * q : 2 * q + 2, ht * P : (ht + 1) * P],
                                x8_t[f"a8{g}"][:, 2 * q : 2 * q + 2, c0 : c0 + cw],
                                start=(q == 0), stop=(q == MT // 2 - 1),
                                perf_mode=DR,
                            )
                        ysb = wk.tile([P, CHUNK], bf16, name=f"y8{g}_{c0}_{ht}", tag="ysb", bufs=4)
                        nc.vector.tensor_mul(
                            ysb[:, :cw], psY[:, :cw], wc_t[g][:, ch + c0 : ch + c0 + cw]
                        )
                        nc.scalar.dma_start(
                            prm[f"y{g}"][:].rearrange("p (ht c) -> p ht c", c=cap)[
                                :, ht, ch + c0 : ch + c0 + cw
                            ],
                            ysb[:, :cw],
                        )

            loop_ctx.close()

    nc.finalize()
    return nc


def _tile_km(w):
    mw = w.shape[1]
    return np.ascontiguousarray(
        w.reshape(KT, P, mw).transpose(1, 0, 2).reshape(P, KT * mw)
    )


def _tile_rhs(w):
    mt = w.shape[0] // P
    return np.ascontiguousarray(
        w.reshape(mt, P, H).transpose(1, 0, 2).reshape(P, mt * H)
    )


def _plan(inputs):
    """Returns (sigs, in_maps, scatter):
    sigs[c] = groups tuple for core c; in_maps[c] = param dict;
    scatter[c] = list of (idx, n) per group (token order hi then lo)."""
    bf = ml_dtypes.bfloat16
    f8 = ml_dtypes.float8_e4m3
    x = np.ascontiguousarray(
        np.asarray(inputs["hidden_states"], dtype=np.float32).reshape(N, H)
    )
    gate_w = np.asarray(inputs["gate_w"], dtype=np.float32)
    Wg = np.asarray(inputs["Wg"], dtype=np.float32)
    Wu = np.asarray(inputs["Wu"], dtype=np.float32)
    Wd = np.asarray(inputs["Wd"], dtype=np.float32)
    sWg = np.asarray(inputs["sWg"], dtype=np.float32)
    sWu = np.asarray(inputs["sWu"], dtype=np.float32)
    sWd = np.asarray(inputs["sWd"], dtype=np.float32)

    logits = x.astype(np.float64) @ gate_w.astype(np.float64).T
    order = np.argsort(-logits, axis=1)
    i1, i2 = order[:, 0], order[:, 1]
    v1 = np.take_along_axis(logits, i1[:, None], 1)[:, 0]
    v2 = np.take_along_axis(logits, i2[:, None], 1)[:, 0]
    ew = np.exp(v2 - v1)
    w1 = 1.0 / (1.0 + ew)
    w2 = ew / (1.0 + ew)

    eidx, ewt = [], []
    for c in range(E):
        sel1, sel2 = i1 == c, i2 == c
        idx = np.nonzero(sel1 | sel2)[0]
        w = np.where(sel1, w1, w2)[idx].astype(np.float32)
        o = np.argsort(-w, kind="stable")
        eidx.append(idx[o])
        ewt.append(w[o])
    counts = np.array([len(ix) for ix in eidx])
    total_w2 = float(w1 @ w1 + w2 @ w2)

    # choose X (bf16 pairs per expert) balancing R vs S core cycles, keeping
    # the fp8 weight-mass fraction under budget
    best = None
    for X in range(120, int(counts.min()) + 1):
        lo = np.sort(counts - X)[::-1]
        L1, L2 = int(lo[0]), int(lo[4])
        cyc_r = (2 * X + 512) * 192
        cyc_s = 512 * 192 + (L1 + L2) * 96
        frac = sum(float(ewt[c][X:] @ ewt[c][X:]) for c in range(E)) / total_w2
        if frac > 0.392:
            continue
        m = max(cyc_r, cyc_s)
        if best is None or m < best[0]:
            best = (m, X, L1, L2, frac)
    if best is None:
        # degenerate routing (extreme imbalance): run every pair in bf16;
        # the fp8 groups become 1-slot dummies computing zeros
        X = int(counts.max())
        best = (0, X, 1, 1, 0.0)
    _, X, L1, L2, frac = best

    lo_counts = counts - X
    lo_order = np.argsort(-lo_counts, kind="stable")

    xT = np.ascontiguousarray(x.T).astype(bf)            # [H, N] bf16
    xT8 = np.ascontiguousarray(x.T * XSCALE).astype(f8)  # [H, N] fp8

    wsets = [(Wg[c], Wu[c], Wd[c]) for c in range(E)]
    wsets.append((sWg[:, :M], sWu[:, :M], sWd[:M, :]))   # shared A
    wsets.append((sWg[:, M:], sWu[:, M:], sWd[M:, :]))   # shared B
    wbf, w8, wdt = {}, {}, {}
    for i, (a, b, d) in enumerate(wsets):
        wdt[i] = _tile_rhs(d.astype(bf))

    def get_bf(i):
        if i not in wbf:
            a, b, d = wsets[i]
            wbf[i] = (_tile_km(a.astype(bf)), _tile_km(b.astype(bf)))
        return wbf[i]

    w8d = {}

    def get_f8d(i):
        if i not in w8d:
            a, b, d = wsets[i]
            w8d[i] = _tile_rhs((d * WSCALE).astype(f8))
        return w8d[i]

    def get_f8(i):
        if i not in w8:
            a, b, d = wsets[i]
            w8[i] = (
                _tile_km((a * WSCALE).astype(f8)),
                _tile_km((b * WSCALE).astype(f8)),
            )
        return w8[i]

    core_specs = []  # per core: list of (wset, cap_hi, cap_lo, idx, w)
    for i in range(4):  # R cores
        ea, eb = 2 * i, 2 * i + 1
        core_specs.append([
            (ea, X, 0, eidx[ea][:X], ewt[ea][:X]),
            (eb, X, 0, eidx[eb][:X], ewt[eb][:X]),
            (9, 512, 0, np.arange(512 * i, 512 * (i + 1)),
             np.ones(512, np.float32)),
        ])
    for i in range(4):  # S cores
        ca, cb = int(lo_order[i]), int(lo_order[7 - i])
        core_specs.append([
            (8, 512, 0, np.arange(512 * i, 512 * (i + 1)),
             np.ones(512, np.float32)),
            (ca, 0, L1, eidx[ca][X:], ewt[ca][X:]),
            (cb, 0, L2, eidx[cb][X:], ewt[cb][X:]),
        ])

    sigs, in_maps, scatter = [], [], []
    for spec in core_specs:
        sig, m, sc = [], {}, []
        for g, (ws, ch, cl, idx, w) in enumerate(spec):
            n = len(idx)
            cap = ch + cl
            assert n <= cap, (n, cap)
            sig.append((ch, cl))
            w_pad = np.zeros(cap, np.float32)
            if ch:
                xe = np.zeros((H, ch), dtype=bf)
                xe[:, :n] = xT[:, idx]
                m[f"x{g}"] = _tile_km(xe)
                tg, tu = get_bf(ws)
                m[f"wg{g}"] = tg
                m[f"wu{g}"] = tu
                w_pad[:n] = w
            else:
                xe = np.zeros((H, cl), dtype=f8)
                xe[:, :n] = xT8[:, idx]
                m[f"x8_{g}"] = _tile_km(xe)
                tg, tu = get_f8(ws)
                m[f"wg8_{g}"] = tg
                m[f"wu8_{g}"] = tu
                w_pad[:n] = w / PSCALE
            if ch:
                m[f"wd{g}"] = wdt[ws]
            if cl:
                m[f"wd8_{g}"] = get_f8d(ws)
            m[f"wc{g}"] = np.ascontiguousarray(
                np.broadcast_to(w_pad[None, :], (P, cap))
            )
            sc.append((idx, n))
        sigs.append(tuple(sig))
        in_maps.append(m)
        scatter.append(sc)
    return sigs, in_maps, scatter


def _prep_in_maps(inputs) -> list:
    return _plan(inputs)[1]


def _unshard(results, sigs, scatter) -> np.ndarray:
    y = np.zeros((N, H), np.float32)
    for c in range(NCORES):
        for g, (idx, n) in enumerate(scatter[c]):
            ch, cl = sigs[c][g]
            cap = ch + cl
            arr = results[c][f"y{g}"].reshape(P, HT, cap)[:, :, :n]
            y[idx] += arr.transpose(2, 1, 0).reshape(n, H).astype(np.float32)
    return y.reshape(B, S, H)


def kernel(**inputs) -> np.ndarray:
    from concourse.bass_utils import run_bass_kernel_spmd

    sigs, in_maps, scatter = _plan(inputs)
    results = [None] * NCORES
    done = set()
    for c in range(NCORES):
        if c in done:
            continue
        cores = [d for d in range(NCORES) if sigs[d] == sigs[c]]
        key = ("v3", sigs[c])
        if key not in _CACHE:
            _CACHE[key] = _build_program(groups=sigs[c])
        res = run_bass_kernel_spmd(
            _CACHE[key], [in_maps[d] for d in cores], list(range(len(cores)))
        ).results
        for j, d in enumerate(cores):
            results[d] = res[j]
            done.add(d)
    return _unshard(results, sigs, scatter)


if __name__ == "__main__":
    sys.path.insert(0, "/root/problem")
    import reference

    inp = reference.setup_inputs()
    expected = np.asarray(reference.reference(**inp))
    actual = kernel(**{k: np.asarray(v) for k, v in inp.items()})
    err = np.linalg.norm(actual - expected) / np.linalg.norm(expected)
    print("Relative error:", err)


# revision 18
# speedup vs baseline: 1.0570x; 1.0001x over previous
"""MoE layer (8 routed experts, top-2, shared experts) on 8 Trainium2 cores,
with selective fp8 (e4m3 DoubleRow) for low-combine-weight routed pairs.

Work layout: the shared expert is split into two virtual experts A/B of
intermediate 512 (combine weight 1), giving 8192 equal-cost token-expert
pairs. Each routed pair's gate/up can run in fp8 at 2x PE throughput; the
quantization error budget allows this only for pairs with small combine
weight, so each expert's pairs are split into its top-X by weight ("hi",
bf16) and the rest ("lo", fp8 gate/up, bf16 down).

Cores 0-3 ("R"): two expert-hi groups of exactly X tokens + a 512-token
sharedB chunk — all bf16. Cores 4-7 ("S"): a 512-token sharedA chunk (bf16)
+ two expert-lo fp8 groups. X is chosen so both core types have equal cycle
counts; shared chunks tile A/B exactly, expert-hi groups are exact by
construction, so padding is limited to the fp8 group capacities.

Matmuls are weights-stationary: gate/up produce [m=128, tokens] in PSUM
(tokens on the free dim — group sizes need no 128-padding), SwiGLU applies
in that layout, and down consumes it directly as stationary — no PE
transposes. fp8 scaling: x8 = e4m3(8x), w8 = e4m3(64W), so PSUM holds 512*g;
Silu rescales by 1/512 and the remaining 512 on the up path is folded into
the per-token combine weight (w/512) applied at the PSUM drain. The Act
engine only runs Silu (no activation-table swaps).
"""

import sys

if "/opt/trn_rl_repo" not in sys.path:
    sys.path.insert(0, "/opt/trn_rl_repo")

import ml_dtypes
import numpy as np

B, S, H = 2, 1024, 2048
N = B * S
E = 8
M = 512
P = 128
KT = H // P
MT = M // P
HT = H // P
NCORES = 8
CHUNK = 512
XSCALE = 8.0      # fp8 activation pre-scale
WSCALE = 64.0     # fp8 weight pre-scale
PSCALE = XSCALE * WSCALE

_CACHE = {}


def _chunks(n):
    if n == 0:
        return []
    k = -(-n // CHUNK)
    base, rem = divmod(n, k)
    out, c0 = [], 0
    for i in range(k):
        w = base + (1 if i < rem else 0)
        out.append((c0, w))
        c0 += w
    return out


def _build_program(collectives=True, loop_n=None, groups=((205, 0), (205, 0), (512, 0))):
    """groups: tuple of (cap_hi, cap_lo) per group."""
    import concourse.mybir as mybir
    import concourse.tile as tile
    from concourse import bacc
    from contextlib import ExitStack

    f32 = mybir.dt.float32
    bf16 = mybir.dt.bfloat16
    f8 = mybir.dt.float8e4
    AF = mybir.ActivationFunctionType
    DR = mybir.MatmulPerfMode.DoubleRow

    nc = bacc.Bacc(None)
    G = len(groups)

    prm = {}
    for g, (ch, cl) in enumerate(groups):
        cap = ch + cl
        if ch:
            prm[f"x{g}"] = nc.declare_dram_parameter(f"x{g}", [P, KT * ch], bf16, isOutput=False)
            prm[f"wg{g}"] = nc.declare_dram_parameter(f"wg{g}", [P, KT * M], bf16, isOutput=False)
            prm[f"wu{g}"] = nc.declare_dram_parameter(f"wu{g}", [P, KT * M], bf16, isOutput=False)
        if cl:
            prm[f"x8_{g}"] = nc.declare_dram_parameter(f"x8_{g}", [P, KT * cl], f8, isOutput=False)
            prm[f"wg8_{g}"] = nc.declare_dram_parameter(f"wg8_{g}", [P, KT * M], f8, isOutput=False)
            prm[f"wu8_{g}"] = nc.declare_dram_parameter(f"wu8_{g}", [P, KT * M], f8, isOutput=False)
        if ch:
            prm[f"wd{g}"] = nc.declare_dram_parameter(f"wd{g}", [P, MT * H], bf16, isOutput=False)
        if cl:
            prm[f"wd8_{g}"] = nc.declare_dram_parameter(f"wd8_{g}", [P, MT * H], f8, isOutput=False)
        prm[f"wc{g}"] = nc.declare_dram_parameter(f"wc{g}", [P, cap], f32, isOutput=False)
        prm[f"y{g}"] = nc.declare_dram_parameter(f"y{g}", [P, HT * cap], bf16, isOutput=True)

    with tile.TileContext(nc) as tc:
        with (
            tc.tile_pool(name="sb", bufs=1) as sb,
            tc.tile_pool(name="io", bufs=2) as io,
            tc.tile_pool(name="wk", bufs=2) as wk,
            tc.tile_pool(name="ps", bufs=6, space="PSUM") as ps,
        ):
            wres = {}
            for g, (ch, cl) in enumerate(groups):
                if ch:
                    t = sb.tile([P, KT, M], bf16, name=f"wg_r{g}")
                    nc.sync.dma_start(t[:], prm[f"wg{g}"][:].rearrange("p (kt m) -> p kt m", m=M))
                    wres[f"wg{g}"] = t
                    t = sb.tile([P, KT, M], bf16, name=f"wu_r{g}")
                    nc.sync.dma_start(t[:], prm[f"wu{g}"][:].rearrange("p (kt m) -> p kt m", m=M))
                    wres[f"wu{g}"] = t
                if cl:
                    t = sb.tile([P, KT, M], f8, name=f"wg8_r{g}")
                    nc.sync.dma_start(t[:], prm[f"wg8_{g}"][:].rearrange("p (kt m) -> p kt m", m=M))
                    wres[f"wg8{g}"] = t
                    t = sb.tile([P, KT, M], f8, name=f"wu8_r{g}")
                    nc.sync.dma_start(t[:], prm[f"wu8_{g}"][:].rearrange("p (kt m) -> p kt m", m=M))
                    wres[f"wu8{g}"] = t
                if ch:
                    t = sb.tile([P, MT, H], bf16, name=f"wd_r{g}")
                    nc.sync.dma_start(t[:], prm[f"wd{g}"][:].rearrange("p (mt h) -> p mt h", h=H))
                    wres[f"wd{g}"] = t
                if cl:
                    t = sb.tile([P, MT, H], f8, name=f"wd8_r{g}")
                    nc.sync.dma_start(t[:], prm[f"wd8_{g}"][:].rearrange("p (mt h) -> p mt h", h=H))
                    wres[f"wd8{g}"] = t

            loop_ctx = ExitStack()
            if loop_n is not None:
                loop_ctx.enter_context(tc.For_i(0, loop_n, 1))

            x_t, x8_t, wc_t, a_t = {}, {}, {}, {}
            for g, (ch, cl) in enumerate(groups):
                cap = ch + cl
                if ch:
                    t = io.tile([P, KT, ch], bf16, name=f"x_t{g}", tag=f"x{g}", bufs=1)
                    nc.sync.dma_start(t[:], prm[f"x{g}"][:].rearrange("p (kt c) -> p kt c", c=ch))
                    x_t[g] = t
                if cl:
                    t = io.tile([P, KT, cl], f8, name=f"x8_t{g}", tag=f"x8{g}", bufs=1)
                    nc.sync.dma_start(t[:], prm[f"x8_{g}"][:].rearrange("p (kt c) -> p kt c", c=cl))
                    x8_t[g] = t
                t = io.tile([P, cap], f32, name=f"wc_t{g}", tag=f"w{g}", bufs=1)
                nc.sync.dma_start(t[:], prm[f"wc{g}"][:])
                wc_t[g] = t
                if ch:
                    a_t[g] = io.tile([P, MT, ch], bf16, name=f"a_t{g}", tag=f"a{g}", bufs=1)
                if cl:
                    x8_t[f"a8{g}"] = io.tile([P, MT, cl], f8, name=f"a8_t{g}", tag=f"a8{g}", bufs=1)

            for g, (ch, cl) in enumerate(groups):
                cap = ch + cl
                # bf16 gate/up chains over hi tokens
                for c0, cw in _chunks(ch):
                    for mt in range(MT):
                        psG = ps.tile([P, CHUNK], f32, name=f"psG{g}_{c0}_{mt}", tag="psG", bufs=3)
                        psU = ps.tile([P, CHUNK], f32, name=f"psU{g}_{c0}_{mt}", tag="psU", bufs=3)
                        for kt in range(KT):
                            nc.tensor.matmul(
                                psG[:, :cw],
                                wres[f"wg{g}"][:, kt, mt * P : (mt + 1) * P],
                                x_t[g][:, kt, c0 : c0 + cw],
                                start=(kt == 0), stop=(kt == KT - 1),
                            )
                            nc.tensor.matmul(
                                psU[:, :cw],
                                wres[f"wu{g}"][:, kt, mt * P : (mt + 1) * P],
                                x_t[g][:, kt, c0 : c0 + cw],
                                start=(kt == 0), stop=(kt == KT - 1),
                            )
                        sil = wk.tile([P, CHUNK], f32, name=f"sil{g}_{c0}_{mt}", tag="sil", bufs=2)
                        nc.scalar.activation(sil[:, :cw], psG[:, :cw], AF.Silu)
                        nc.vector.tensor_mul(
                            a_t[g][:, mt, c0 : c0 + cw], sil[:, :cw], psU[:, :cw]
                        )
                # fp8 DoubleRow gate/up chains over lo tokens (a cols offset ch)
                for c0, cw in _chunks(cl):
                    for mt in range(MT):
                        psG = ps.tile([P, CHUNK], f32, name=f"ps8G{g}_{c0}_{mt}", tag="psG", bufs=3)
                        psU = ps.tile([P, CHUNK], f32, name=f"ps8U{g}_{c0}_{mt}", tag="psU", bufs=3)
                        for q in range(KT // 2):
                            nc.tensor.matmul(
                                psG[:, :cw],
                                wres[f"wg8{g}"][:, 2 * q : 2 * q + 2, mt * P : (mt + 1) * P],
                                x8_t[g][:, 2 * q : 2 * q + 2, c0 : c0 + cw],
                                start=(q == 0), stop=(q == KT // 2 - 1),
                                perf_mode=DR,
                            )
                            nc.tensor.matmul(
                                psU[:, :cw],
                                wres[f"wu8{g}"][:, 2 * q : 2 * q + 2, mt * P : (mt + 1) * P],
                                x8_t[g][:, 2 * q : 2 * q + 2, c0 : c0 + cw],
                                start=(q == 0), stop=(q == KT // 2 - 1),
                                perf_mode=DR,
                            )
                        sil = wk.tile([P, CHUNK], f32, name=f"sil8{g}_{c0}_{mt}", tag="sil", bufs=2)
                        nc.scalar.activation(sil[:, :cw], psG[:, :cw], AF.Silu, scale=1.0 / PSCALE)
                        nc.vector.scalar_tensor_tensor(
                            x8_t[f"a8{g}"][:, mt, c0 : c0 + cw],
                            psU[:, :cw], 1.0 / WSCALE, sil[:, :cw],
                            mybir.AluOpType.mult, mybir.AluOpType.mult,
                        )
                # down projection over the combined token range
                for c0, cw in _chunks(ch):
                    for ht in range(HT):
                        psY = ps.tile([P, CHUNK], f32, name=f"psY{g}_{c0}_{ht}", tag="psY", bufs=2)
                        for mt in range(MT):
                            nc.tensor.matmul(
                                psY[:, :cw],
                                wres[f"wd{g}"][:, mt, ht * P : (ht + 1) * P],
                                a_t[g][:, mt, c0 : c0 + cw],
                                start=(mt == 0), stop=(mt == MT - 1),
                            )
                        ysb = wk.tile([P, CHUNK], bf16, name=f"y{g}_{c0}_{ht}", tag="ysb", bufs=4)
                        nc.vector.tensor_mul(
                            ysb[:, :cw], psY[:, :cw], wc_t[g][:, c0 : c0 + cw]
                        )
                        nc.scalar.dma_start(
                            prm[f"y{g}"][:].rearrange("p (ht c) -> p ht c", c=cap)[
                                :, ht, c0 : c0 + cw
                            ],
                            ysb[:, :cw],
                        )
                for c0, cw in _chunks(cl):
                    for ht in range(HT):
                        psY = ps.tile([P, CHUNK], f32, name=f"psY8{g}_{c0}_{ht}", tag="psY", bufs=2)
                        for q in range(MT // 2):
                            nc.tensor.matmul(
                                psY[:, :cw],
                                wres[f"wd8{g}"][:, 2 * q : 2 * q + 2, ht * P : (ht + 1) * P],
                                x8_t[f"a8{g}"][:, 2 * q : 2 * q + 2, c0 : c0 + cw],
                                start=(q == 0), stop=(q == MT // 2 - 1),
                                perf_mode=DR,
                            )
                        ysb = wk.tile([P, CHUNK], bf16, name=f"y8{g}_{c0}_{ht}", tag="ysb", bufs=4)
                        nc.vector.tensor_mul(
                            ysb[:, :cw], psY[:, :cw], wc_t[g][:, ch + c0 : ch + c0 + cw]
                        )
                        nc.scalar.dma_start(
                            prm[f"y{g}"][:].rearrange("p (ht c) -> p ht c", c=cap)[
                                :, ht, ch + c0 : ch + c0 + cw
                            ],
                            ysb[:, :cw],
                        )

            loop_ctx.close()

    nc.finalize()
    return nc


def _tile_km(w):
    mw = w.shape[1]
    return np.ascontiguousarray(
        w.reshape(KT, P, mw).transpose(1, 0, 2).reshape(P, KT * mw)
    )


def _tile_rhs(w):
    mt = w.shape[0] // P
    return np.ascontiguousarray(
        w.reshape(mt, P, H).transpose(1, 0, 2).reshape(P, mt * H)
    )


def _plan(inputs):
    """Returns (sigs, in_maps, scatter):
    sigs[c] = groups tuple for core c; in_maps[c] = param dict;
    scatter[c] = list of (idx, n) per group (token order hi then lo)."""
    bf = ml_dtypes.bfloat16
    f8 = ml_dtypes.float8_e4m3
    x = np.ascontiguousarray(
        np.asarray(inputs["hidden_states"], dtype=np.float32).reshape(N, H)
    )
    gate_w = np.asarray(inputs["gate_w"], dtype=np.float32)
    Wg = np.asarray(inputs["Wg"], dtype=np.float32)
    Wu = np.asarray(inputs["Wu"], dtype=np.float32)
    Wd = np.asarray(inputs["Wd"], dtype=np.float32)
    sWg = np.asarray(inputs["sWg"], dtype=np.float32)
    sWu = np.asarray(inputs["sWu"], dtype=np.float32)
    sWd = np.asarray(inputs["sWd"], dtype=np.float32)

    logits = x.astype(np.float64) @ gate_w.astype(np.float64).T
    order = np.argsort(-logits, axis=1)
    i1, i2 = order[:, 0], order[:, 1]
    v1 = np.take_along_axis(logits, i1[:, None], 1)[:, 0]
    v2 = np.take_along_axis(logits, i2[:, None], 1)[:, 0]
    ew = np.exp(v2 - v1)
    w1 = 1.0 / (1.0 + ew)
    w2 = ew / (1.0 + ew)

    eidx, ewt = [], []
    for c in range(E):
        sel1, sel2 = i1 == c, i2 == c
        idx = np.nonzero(sel1 | sel2)[0]
        w = np.where(sel1, w1, w2)[idx].astype(np.float32)
        o = np.argsort(-w, kind="stable")
        eidx.append(idx[o])
        ewt.append(w[o])
    counts = np.array([len(ix) for ix in eidx])
    total_w2 = float(w1 @ w1 + w2 @ w2)

    # choose X (bf16 pairs per expert) balancing R vs S core cycles, keeping
    # the fp8 weight-mass fraction under budget
    best = None
    for X in range(120, int(counts.min()) + 1):
        lo = np.sort(counts - X)[::-1]
        L1, L2 = int(lo[0]), int(lo[4])
        cyc_r = (2 * X + 512) * 192
        cyc_s = 512 * 192 + (L1 + L2) * 96
        frac = sum(float(ewt[c][X:] @ ewt[c][X:]) for c in range(E)) / total_w2
        if frac > 0.392:
            continue
        m = max(cyc_r, cyc_s)
        if best is None or m < best[0]:
            best = (m, X, L1, L2, frac)
    if best is None:
        # degenerate routing (extreme imbalance): run every pair in bf16;
        # the fp8 groups become 1-slot dummies computing zeros
        X = int(counts.max())
        best = (0, X, 1, 1, 0.0)
    _, X, L1, L2, frac = best

    lo_counts = counts - X
    lo_order = np.argsort(-lo_counts, kind="stable")

    xT = np.ascontiguousarray(x.T).astype(bf)            # [H, N] bf16
    xT8 = np.ascontiguousarray(x.T * XSCALE).astype(f8)  # [H, N] fp8

    wsets = [(Wg[c], Wu[c], Wd[c]) for c in range(E)]
    wsets.append((sWg[:, :M], sWu[:, :M], sWd[:M, :]))   # shared A
    wsets.append((sWg[:, M:], sWu[:, M:], sWd[M:, :]))   # shared B
    wbf, w8, wdt = {}, {}, {}
    for i, (a, b, d) in enumerate(wsets):
        wdt[i] = _tile_rhs(d.astype(bf))

    def get_bf(i):
        if i not in wbf:
            a, b, d = wsets[i]
            wbf[i] = (_tile_km(a.astype(bf)), _tile_km(b.astype(bf)))
        return wbf[i]

    w8d = {}

    def get_f8d(i):
        if i not in w8d:
            a, b, d = wsets[i]
            w8d[i] = _tile_rhs((d * WSCALE).astype(f8))
        return w8d[i]

    def get_f8(i):
        if i not in w8:
            a, b, d = wsets[i]
            w8[i] = (
                _tile_km((a * WSCALE).astype(f8)),
                _tile_km((b * WSCALE).astype(f8)),
            )
        return w8[i]

    core_specs = []  # per core: list of (wset, cap_hi, cap_lo, idx, w)
    for i in range(4):  # R cores
        ea, eb = 2 * i, 2 * i + 1
        core_specs.append([
            (ea, X, 0, eidx[ea][:X], ewt[ea][:X]),
            (eb, X, 0, eidx[eb][:X], ewt[eb][:X]),
            (9, 512, 0, np.arange(512 * i, 512 * (i + 1)),
             np.ones(512, np.float32)),
        ])
    for i in range(4):  # S cores
        ca, cb = int(lo_order[i]), int(lo_order[7 - i])
        core_specs.append([
            (8, 512, 0, np.arange(512 * i, 512 * (i + 1)),
             np.ones(512, np.float32)),
            (ca, 0, L1, eidx[ca][X:], ewt[ca][X:]),
            (cb, 0, L2, eidx[cb][X:], ewt[cb][X:]),
        ])

    sigs, in_maps, scatter = [], [], []
    for spec in core_specs:
        sig, m, sc = [], {}, []
        for g, (ws, ch, cl, idx, w) in enumerate(spec):
            n = len(idx)
            cap = ch + cl
            assert n <= cap, (n, cap)
            sig.append((ch, cl))
            w_pad = np.zeros(cap, np.float32)
            if ch:
                xe = np.zeros((H, ch), dtype=bf)
                xe[:, :n] = xT[:, idx]
                m[f"x{g}"] = _tile_km(xe)
                tg, tu = get_bf(ws)
                m[f"wg{g}"] = tg
                m[f"wu{g}"] = tu
                w_pad[:n] = w
            else:
                xe = np.zeros((H, cl), dtype=f8)
                xe[:, :n] = xT8[:, idx]
                m[f"x8_{g}"] = _tile_km(xe)
                tg, tu = get_f8(ws)
                m[f"wg8_{g}"] = tg
                m[f"wu8_{g}"] = tu
                w_pad[:n] = w / PSCALE
            if ch:
                m[f"wd{g}"] = wdt[ws]
            if cl:
                m[f"wd8_{g}"] = get_f8d(ws)
            m[f"wc{g}"] = np.ascontiguousarray(
                np.broadcast_to(w_pad[None, :], (P, cap))
            )
            sc.append((idx, n))
        sigs.append(tuple(sig))
        in_maps.append(m)
        scatter.append(sc)
    return sigs, in_maps, scatter


def _prep_in_maps(inputs) -> list:
    return _plan(inputs)[1]


def _unshard(results, sigs, scatter) -> np.ndarray:
    y = np.zeros((N, H), np.float32)
    for c in range(NCORES):
        for g, (idx, n) in enumerate(scatter[c]):
            ch, cl = sigs[c][g]
            cap = ch + cl
            arr = results[c][f"y{g}"].reshape(P, HT, cap)[:, :, :n]
            y[idx] += arr.transpose(2, 1, 0).reshape(n, H).astype(np.float32)
    return y.reshape(B, S, H)


def kernel(**inputs) -> np.ndarray:
    from concourse.bass_utils import run_bass_kernel_spmd

    sigs, in_maps, scatter = _plan(inputs)
    results = [None] * NCORES
    done = set()
    for c in range(NCORES):
        if c in done:
            continue
        cores = [d for d in range(NCORES) if sigs[d] == sigs[c]]
        key = ("v3", sigs[c])
        if key not in _CACHE:
            _CACHE[key] = _build_program(groups=sigs[c])
        res = run_bass_kernel_spmd(
            _CACHE[key], [in_maps[d] for d in cores], list(range(len(cores)))
        ).results
        for j, d in enumerate(cores):
            results[d] = res[j]
            done.add(d)
    return _unshard(results, sigs, scatter)


if __name__ == "__main__":
    sys.path.insert(0, "/root/problem")
    import reference

    inp = reference.setup_inputs()
    expected = np.asarray(reference.reference(**inp))
    actual = kernel(**{k: np.asarray(v) for k, v in inp.items()})
    err = np.linalg.norm(actual - expected) / np.linalg.norm(expected)
    print("Relative error:", err)
